# revision 1
# baseline (speedup 1.0000x reference)
"""Trainium2 Bass kernel for nn_CircumpunctAttention_17136919511703.

Sharding: 16 heads tensor-parallel over 8 cores (2 heads/core); W_out
row-parallel with the cross-core partial sum done on the host.

Math simplifications (validated to ~2e-7 abs err vs the jax reference,
output absmax ~0.03):
  - attn = softmax(s)*ap*ex renormalized by (sum + 1e-8): the aperture
    gate ap cancels exactly up to the 1e-8 term (relative ~4e-8), so it
    is dropped entirely.  converged = (e' @ v) / (e' @ 1) with
    e'_st = exp(scoresT_st + ln(ex_s)) (sender gate folded into the exp
    bias, per-partition on the ACT engine).
  - no softmax max-subtraction (scores are bounded, |s| < 4).
  - the per-head "aperture chamber" (valve in/out, phase rotation, chi)
    is a per-head linear map on the head dim -> folded into W_out on the
    host: W'_h = c_h * Wout_h @ R_h.

Per-core dataflow (all matmuls fp32r):
  xT [D,T] (host-transposed) -> innerT/outerT/vT [128,T] + ex logits
  [2,T] projections; v transposed back to natural via PE transposes to
  build vpp=[v|1] [128, t_tile, head, 65]; per head: scoresT [s,t] in
  PSUM -> ACT exp(+lnex bias) -> eT SBUF -> PT accumulation
  [65(=64+E row), T] -> f=1/E -> conv scaled via a K=1 broadcast matmul
  -> final y_partial = convT.T @ wpT -> DMA out.  Host sums 8 partials.
"""

import math
import os
from contextlib import ExitStack

import numpy as np

import concourse.bass as bass
import concourse.mybir as mybir
import concourse.tile as tile
from concourse import bacc
from concourse.bass_utils import run_bass_kernel_spmd
from concourse.masks import make_identity

T, D, H, DH = 2048, 1024, 16, 64
NCORES = 8
HPC = H // NCORES          # heads per core = 2
KW = HPC * DH              # per-core head width = 128
SCALE = math.sqrt(DH)
DT = D // 128              # d tiles = 8
TT = T // 128              # t/s tiles = 16
CH = T // 512              # 512-col chunks over T = 4
F32 = mybir.dt.float32
F32R = mybir.dt.float32r
AF = mybir.ActivationFunctionType

_CACHE = {}
LAST_RESULTS = None


def _build_nc():
    nc = bacc.Bacc()
    xT = nc.declare_dram_parameter("xT", [128, DT, T], F32R, isOutput=False)
    wiT = nc.declare_dram_parameter("wiT", [128, DT, KW], F32R, isOutput=False)
    woT = nc.declare_dram_parameter("woT", [128, DT, KW], F32R, isOutput=False)
    wvT = nc.declare_dram_parameter("wvT", [128, DT, KW], F32R, isOutput=False)
    weT = nc.declare_dram_parameter("weT", [128, DT, HPC], F32R, isOutput=False)
    webn = nc.declare_dram_parameter("webn", [128, TT, HPC], F32, isOutput=False)
    wpT = nc.declare_dram_parameter("wpT", [KW, D], F32R, isOutput=False)
    ones2 = nc.declare_dram_parameter("ones2", [128, 64], F32R, isOutput=False)
    y = nc.declare_dram_parameter("y", [T, D], F32, isOutput=True)

    with tile.TileContext(nc) as tc, ExitStack() as ctx:
        _body(ctx, tc, xT, wiT, woT, wvT, weT, webn, wpT, ones2, y)
    nc.compile()
    return nc


def _body(ctx, tc, xT, wiT, woT, wvT, weT, webn, wpT, ones2, y):
    nc = tc.nc
    P = 128
    HT = 1024  # t-half width

    const = ctx.enter_context(tc.tile_pool(name="const", bufs=1))
    persist = ctx.enter_context(tc.tile_pool(name="persist", bufs=1))
    eTp = ctx.enter_context(tc.tile_pool(name="eTp", bufs=10))
    stage = ctx.enter_context(tc.tile_pool(name="stage", bufs=2))
    # single uniform PSUM pool: 4 slots x [128, 1024] (2 banks each)
    psum = ctx.enter_context(tc.tile_pool(name="psum", bufs=4, space="PSUM"))

    def ps_tile(name="psm"):
        return psum.tile([P, HT], F32, tag="m", name=name)

    ident = const.tile([P, P], F32)
    make_identity(nc, ident)
    ones_k1 = const.tile([1, 64], F32R)
    nc.sync.dma_start(out=ones_k1, in_=ones2[0:1, :])
    onep = const.tile([P, 1], F32)
    nc.vector.memset(onep, 1.0)
    bnat = const.tile([P, TT, HPC], F32)
    nc.sync.dma_start(out=bnat, in_=webn[:, :, :])

    # weights (d on partitions): [p, d_tile, k]
    wiT_sb = const.tile([P, DT, KW], F32R)
    woT_sb = const.tile([P, DT, KW], F32R)
    wvT_sb = const.tile([P, DT, KW], F32R)
    weT_sb = const.tile([P, DT, HPC], F32R)
    nc.sync.dma_start(out=wiT_sb, in_=wiT[:, :, :])
    nc.sync.dma_start(out=woT_sb, in_=woT[:, :, :])
    nc.sync.dma_start(out=wvT_sb, in_=wvT[:, :, :])
    nc.sync.dma_start(out=weT_sb, in_=weT[:, :, :])
    wpT_sb = const.tile([KW, D], F32R)
    nc.sync.dma_start(out=wpT_sb, in_=wpT[:, :])

    xT_sb = persist.tile([P, DT, T], F32R)
    for a in range(DT):
        nc.sync.dma_start(out=xT_sb[:, a, :], in_=xT[:, a, :])

    innerT = persist.tile([P, T], F32R)
    outerT = persist.tile([P, T], F32R)
    vT = persist.tile([P, T], F32)
    exlT = persist.tile([HPC, T], F32)
    vpp = persist.tile([P, TT, HPC, 65], F32R)
    nc.sync.dma_start(
        out=vpp[:, :, :, 64],
        in_=ones2[:, 0:TT * HPC].rearrange("p (a b) -> p a b", a=TT))
    ex_nat = persist.tile([P, TT, HPC], F32)
    convT = persist.tile([KW, T], F32R)

    def xchunk(a, half, j2):
        base = half * HT + j2 * 512
        return xT_sb[:, a, base:base + 512].bitcast(F32R)

    # ---- projection wave 1 (a-major, all 4 psum slots in flight so the
    # PE consumes each xT d-tile as its DMA lands) ----
    psA, psB = ps_tile("ex0"), ps_tile("v0")
    ps1, ps2 = ps_tile("i0"), ps_tile("o0")
    for a in range(DT):
        st, sp_ = (a == 0), (a == DT - 1)
        for j2 in range(2):
            nc.tensor.matmul(
                psA[0:HPC, j2 * 512:(j2 + 1) * 512],
                lhsT=weT_sb[:, a, :].bitcast(F32R),
                rhs=xchunk(a, 0, j2), start=st, stop=sp_)
            nc.tensor.matmul(
                psB[:, j2 * 512:(j2 + 1) * 512],
                lhsT=wvT_sb[:, a, :].bitcast(F32R),
                rhs=xchunk(a, 0, j2), start=st, stop=sp_)
            nc.tensor.matmul(
                ps1[:, j2 * 512:(j2 + 1) * 512],
                lhsT=wiT_sb[:, a, :].bitcast(F32R),
                rhs=xchunk(a, 0, j2), start=st, stop=sp_)
            nc.tensor.matmul(
                ps2[:, j2 * 512:(j2 + 1) * 512],
                lhsT=woT_sb[:, a, :].bitcast(F32R),
                rhs=xchunk(a, 0, j2), start=st, stop=sp_)
    nc.scalar.copy(out=exlT[:, 0:HT], in_=psA[0:HPC, :])
    nc.scalar.copy(out=vT[:, 0:HT], in_=psB)
    nc.vector.tensor_copy(out=innerT[:, 0:HT], in_=ps1)
    nc.vector.tensor_copy(out=outerT[:, 0:HT], in_=ps2)

    def emit_proj(wsb, half, rows=P, name="pw"):
        ps = ps_tile(name)
        for a in range(DT):
            for j2 in range(2):
                nc.tensor.matmul(
                    ps[0:rows, j2 * 512:(j2 + 1) * 512],
                    lhsT=wsb[:, a, :].bitcast(F32R),
                    rhs=xchunk(a, half, j2),
                    start=(a == 0), stop=(a == DT - 1))
        return ps

    def emit_sig(half):
        # ex_nat[:, i, h] = sigmoid(z + b), Exp-only (one ACT table set)
        ps = ps_tile("sig")
        for k in range(8):
            i = half * 8 + k
            nc.tensor.transpose(
                ps[:, k * 64:k * 64 + HPC], exlT[:, i * P:(i + 1) * P],
                ident[0:HPC, 0:HPC])
        sl = slice(half * 8, (half + 1) * 8)
        nc.vector.tensor_add(
            out=ex_nat[:, sl, :],
            in0=ps.rearrange("p (i c) -> p i c", c=64)[:, 0:8, 0:HPC],
            in1=bnat[:, sl, :])
        nc.scalar.activation(out=ex_nat[:, sl, :], in_=ex_nat[:, sl, :],
                             func=AF.Exp, scale=-1.0)
        nc.vector.tensor_scalar(
            out=ex_nat[:, sl, :], in0=ex_nat[:, sl, :],
            scalar1=onep, scalar2=None, op0=mybir.AluOpType.add)
        nc.vector.reciprocal(out=ex_nat[:, sl, :], in_=ex_nat[:, sl, :])

    def emit_vppg(g):
        # vpp[:, i, h, :] = [v_i_h | 1] * ex_nat[:, i, h]
        ps = ps_tile("vtr")
        for k in range(8):
            i = g * 8 + k
            nc.tensor.transpose(
                ps[:, k * P:(k + 1) * P], vT[:, i * P:(i + 1) * P], ident)
        psr = ps.rearrange("p (k c) -> p k c", c=P)
        for h in range(HPC):
            nc.vector.tensor_copy(
                out=vpp[:, g * 8:(g + 1) * 8, h, 0:DH],
                in_=psr[:, :, h * DH:(h + 1) * DH])
        for k in range(8):
            i = g * 8 + k
            for h in range(HPC):
                nc.vector.tensor_scalar_mul(
                    out=vpp[:, i, h, :], in0=vpp[:, i, h, :],
                    scalar1=ex_nat[:, i, h:h + 1])

    emit_sig(0)
    emit_vppg(0)

    convT = persist.tile([KW, T], F32R)

    # ---- attention, both heads packed per i: scores h0/h1 run in PE row
    # groups (0,0)/(64,0) into one psum tile, one wide exp covers both ----
    def emit_attn2(i, jj, pt0, pt1):
        for j2 in range(2):
            j = jj * 2 + j2
            sc = ps_tile("sc")
            nc.tensor.matmul(
                sc[:, 0:512],
                lhsT=outerT[0:DH, i * P:(i + 1) * P].bitcast(F32R),
                rhs=innerT[0:DH, j * 512:(j + 1) * 512].bitcast(F32R),
                start=True, stop=True)
            nc.tensor.matmul(
                sc[:, 512:HT],
                lhsT=outerT[DH:KW, i * P:(i + 1) * P].bitcast(F32R),
                rhs=innerT[DH:KW, j * 512:(j + 1) * 512].bitcast(F32R),
                start=True, stop=True)
            eT = eTp.tile([P, HT], F32R, tag="e", name="eT")
            nc.scalar.activation(out=eT, in_=sc, func=AF.Exp, scale=1.0)
            nc.tensor.matmul(
                pt0[0:65, j2 * 512:(j2 + 1) * 512],
                lhsT=vpp[:, i, 0, :].bitcast(F32R),
                rhs=eT[:, 0:512].bitcast(F32R),
                start=(i == 0), stop=(i == TT - 1))
            nc.tensor.matmul(
                pt1[0:65, j2 * 512:(j2 + 1) * 512],
                lhsT=vpp[:, i, 1, :].bitcast(F32R),
                rhs=eT[:, 512:HT].bitcast(F32R),
                start=(i == 0), stop=(i == TT - 1))

    def emit_post(h, jj, pt):
        # conv[:, half] = PT[0:64] / E ; via f=1/E broadcast K=1 matmul
        f_row = stage.tile([HPC, HT], F32R, tag="sp", name="f_row")[0:1, :]
        with nc.allow_low_precision(reason="f32r is a 4-byte container"):
            nc.vector.reciprocal(out=f_row, in_=pt[64:65, 0:HT])
        p_sb = stage.tile([DH, HT], F32, tag="P")
        nc.vector.tensor_copy(out=p_sb[:, 0:512], in_=pt[0:DH, 0:512])
        nc.scalar.copy(out=p_sb[:, 512:HT], in_=pt[0:DH, 512:HT])
        fps = ps_tile("fps")
        for j2 in range(2):
            nc.tensor.matmul(
                fps[0:DH, j2 * 512:(j2 + 1) * 512],
                lhsT=ones_k1.bitcast(F32R),
                rhs=f_row[:, j2 * 512:(j2 + 1) * 512].bitcast(F32R),
                start=True, stop=True)
        nc.vector.tensor_mul(
            out=convT[h * DH:(h + 1) * DH, jj * HT:(jj + 1) * HT],
            in0=p_sb, in1=fps[0:DH, :])

    def emit_final_slice(m, dve_only=False):
        py = ps_tile("py")
        for o in range(2):
            nc.tensor.matmul(
                py[:, o * 512:(o + 1) * 512],
                lhsT=convT[:, m * P:(m + 1) * P].bitcast(F32R),
                rhs=wpT_sb[:, o * 512:(o + 1) * 512].bitcast(F32R),
                start=True, stop=True)
        y_sb = stage.tile([P, D], F32, tag="y", bufs=3, name="y_sb")
        if dve_only or m % 2 == 0:
            nc.vector.tensor_copy(out=y_sb, in_=py)
        else:
            nc.scalar.copy(out=y_sb, in_=py)
        nc.sync.dma_start(out=y[m * P:(m + 1) * P, :], in_=y_sb)

    # wave-2 groups, interleaved into the first attention phase
    def w2_ex1():
        ps = emit_proj(weT_sb, 1, rows=HPC, name="ex1")
        nc.scalar.copy(out=exlT[:, HT:T], in_=ps[0:HPC, :])
        emit_sig(1)

    def w2_v1():
        ps = emit_proj(wvT_sb, 1, name="v1")
        nc.scalar.copy(out=vT[:, HT:T], in_=ps)
        emit_vppg(1)

    def w2_o1():
        ps = emit_proj(woT_sb, 1, name="o1")
        nc.vector.tensor_copy(out=outerT[:, HT:T], in_=ps)

    def w2_i1():
        ps = emit_proj(wiT_sb, 1, name="i1")
        nc.vector.tensor_copy(out=innerT[:, HT:T], in_=ps)

    wave2 = [w2_ex1, w2_v1, w2_o1, w2_i1]

    # jj=0 for both heads; wave-2 trickles through the spare psum slot
    pt00 = psum.tile([P, HT], F32, tag="m", name="pt00")
    pt10 = psum.tile([P, HT], F32, tag="m", name="pt10")
    for i in range(8):
        emit_attn2(i, 0, pt00, pt10)
        if i % 2 == 1:
            wave2[i // 2]()
    for i in range(8, TT):
        emit_attn2(i, 0, pt00, pt10)
    emit_post(0, 0, pt00)
    emit_post(1, 0, pt10)

    # jj=1 for both heads; final slices of t-half 0 interleave in
    pt01 = psum.tile([P, HT], F32, tag="m", name="pt01")
    pt11 = psum.tile([P, HT], F32, tag="m", name="pt11")
    for i in range(TT):
        emit_attn2(i, 1, pt01, pt11)
        if 2 <= i < 10:
            emit_final_slice(i - 2, dve_only=True)
    emit_post(0, 1, pt01)
    emit_post(1, 1, pt11)
    for m in range(8, TT):
        emit_final_slice(m)


def _sigmoid(z):
    return 1.0 / (1.0 + np.exp(-z))


def _prep_in_maps(inputs):
    x = np.ascontiguousarray(np.asarray(inputs["x"], np.float32)[0])  # [T, D]
    xT = np.ascontiguousarray(x.T)                                    # [D, T]
    Wi = np.asarray(inputs["Wi_w"], np.float32).reshape(H, DH, D) / SCALE
    Wo = np.asarray(inputs["Wo_w"], np.float32).reshape(H, DH, D)
    Wv = np.asarray(inputs["Wv_w"], np.float32).reshape(H, DH, D)
    We = np.asarray(inputs["We_w"], np.float32)                       # [H, D]
    We_b = np.asarray(inputs["We_b"], np.float32)                     # [H]
    Wout = np.asarray(inputs["Wout_w"], np.float32)                   # [D, D]
    beta = np.asarray(inputs["beta"], np.float32)
    iv = np.asarray(inputs["iv"], np.float32)
    ov = np.asarray(inputs["ov"], np.float32)
    chi = np.asarray(inputs["chi"], np.float32)

    # chamber folded into Wout: W'_h = c_h * Wout_h @ R_h
    ang = np.float32(math.pi) * _sigmoid(beta)
    c_h = _sigmoid(iv) * _sigmoid(ov) * np.tanh(chi)                  # [H]
    cos_a, sin_a = np.cos(ang), np.sin(ang)
    HALF = DH // 2
    Wp = np.zeros((H, D, DH), np.float32)
    for h in range(H):
        Wh = Wout[:, h * DH:(h + 1) * DH]
        Wp[h][:, :HALF] = c_h[h] * (Wh[:, :HALF] * cos_a[h] + Wh[:, HALF:] * sin_a[h])
        Wp[h][:, HALF:] = c_h[h] * (-Wh[:, :HALF] * sin_a[h] + Wh[:, HALF:] * cos_a[h])

    def dtile(arr):  # [D, X] -> [128, DT, X] (d-tile-major, partition-contig)
        return np.ascontiguousarray(
            arr.reshape(DT, 128, arr.shape[1]).transpose(1, 0, 2))

    xTr = dtile(xT)
    ones2 = np.ones((128, 64), np.float32)
    in_maps = []
    for c in range(NCORES):
        hs = slice(HPC * c, HPC * (c + 1))
        wiT = dtile(Wi[hs].reshape(KW, D).T)
        woT = dtile(Wo[hs].reshape(KW, D).T)
        wvT = dtile(Wv[hs].reshape(KW, D).T)
        weT = dtile(We[hs].T)
        webn = np.ascontiguousarray(
            np.broadcast_to(We_b[hs], (128, TT, HPC)).astype(np.float32))
        wpT = np.ascontiguousarray(
            Wp[hs].transpose(0, 2, 1).reshape(KW, D))                 # [128, D]
        in_maps.append(dict(xT=xTr, wiT=wiT, woT=woT, wvT=wvT,
                            weT=weT, webn=webn, wpT=wpT, ones2=ones2))
    return in_maps


def kernel(**inputs):
    global LAST_RESULTS
    if "nc" not in _CACHE:
        _CACHE["nc"] = _build_nc()
    nc = _CACHE["nc"]
    in_maps = _prep_in_maps(inputs)
    trace = bool(os.environ.get("CIRC_TRACE"))
    res = run_bass_kernel_spmd(
        nc, in_maps, core_ids=list(range(NCORES)), trace=trace)
    LAST_RESULTS = res
    y = res.results[0]["y"].astype(np.float32)
    for c in range(1, NCORES):
        y = y + res.results[c]["y"]
    return y.reshape(1, T, D)

